# revision 5
# baseline (speedup 1.0000x reference)
"""BlockDiagonalLowRankLinear Trainium2 kernel.

y = BlockDiag(blocks) @ x + U @ (V.T @ x), scaled by alpha, plus bias.

Shapes (full problem):
  x      [4, 2048, 4096] f32   -> flattened to [8192, 4096]
  blocks [16, 256, 256]  f32   (per-block [out, in])
  U      [4096, 64] f32, V [4096, 64] f32, bias [4096] f32, alpha [1] f32
  out    [4, 2048, 4096] f32

Sharding: data-parallel over tokens. Each of the 8 cores gets 1024 tokens
and the full (replicated) parameters; outputs are concatenated. No
collectives needed.

The baseline f32 version was DMA-bound (~40MB/core ~= 120us at ~330GB/s).
This version moves all device I/O to bf16 (host converts; rel err ~5e-3
vs the 2e-2 gate) and restructures host-side so the device does nothing
but matmuls:
  - alpha is folded into blocks and U on the host.
  - blocks^T and U^T are pre-transposed on the host into the PE chunk
    layout; bias is appended as a 65th row of U^T so the bias add happens
    inside the low-rank matmul (K=65) for free.
  - x^T is produced by the DMA XBAR transpose (16x128 tiles, bf16) during
    the load - no PE/DVE transposes at all.

Per-core per-slab (256 tokens) steady state:
  DMA: x^T slab load (2 transpose-DMAs), 2 out row-block writes.
  PE : 32 V^T x matmuls (N=256) for the NEXT slab interleaved with
       8x2 accumulation groups of 4 block-diag matmuls (N=256) + 1
       low-rank+bias matmul (K=65, N=512) for THIS slab.
  ACT/DVE: PSUM->SBUF bf16 copies of outputs; DVE copies t_lr to SBUF.
"""

import numpy as np
import ml_dtypes

import concourse.bacc as bacc
import concourse.bass as bass
import concourse.mybir as mybir
import concourse.tile as tile
from concourse.bass_utils import run_bass_kernel_spmd

F32 = mybir.dt.float32
BF16 = mybir.dt.bfloat16
NPBF = ml_dtypes.bfloat16

N_CORES = 8
D = 4096          # in = out features
R = 64            # low rank
NB = 16           # diagonal blocks
NK = D // 128     # 32 i-chunks of 128
T_CORE = 1024     # tokens per core
T_SLAB = 256      # tokens per slab
OC = 512          # output column chunk (one PSUM bank)


def build(t_core: int = T_CORE, repeats: int = 1):
    nc = bacc.Bacc("TRN2", target_bir_lowering=False, debug=False)
    x = nc.declare_dram_parameter("x", [t_core, D], BF16, isOutput=False)
    bt = nc.declare_dram_parameter("bt", [128, NK, 256], BF16, isOutput=False)
    uaug = nc.declare_dram_parameter("uaug", [R + 1, NK, 128], BF16, isOutput=False)
    v = nc.declare_dram_parameter("v", [128, NK, R], BF16, isOutput=False)
    out = nc.declare_dram_parameter("out", [t_core, D], BF16, isOutput=True)

    n_slab = t_core // T_SLAB
    n_tc = T_SLAB // 128          # token chunks per slab
    n_oc = D // OC                # 8 output chunks

    with tile.TileContext(nc) as tc:
        with (
            tc.tile_pool(name="const", bufs=1) as cpool,
            tc.tile_pool(name="psum", bufs=5, space="PSUM") as psum,
            tc.tile_pool(name="lrpsum", bufs=2, space="PSUM") as lrpsum,
            tc.tile_pool(name="xT", bufs=2) as xTpool,
            tc.tile_pool(name="tlr", bufs=2) as tlrpool,
            tc.tile_pool(name="opool", bufs=3) as opool,
        ):
            xT_tiles = [None] * n_slab
            tlr_tiles = [None] * n_slab
            tlr_sb_tiles = [None] * n_slab

            def load_xT(s):
                """Transpose-load slab s of x into a fresh xT tile."""
                t0 = (s % n_slab) * T_SLAB
                xt = xTpool.tile([128, NK, T_SLAB], BF16, tag="xT")
                for h in range(2):
                    nc.sync.dma_start(
                        xt[:, h * (NK // 2):(h + 1) * (NK // 2), :],
                        x[t0:t0 + T_SLAB, h * (D // 2):(h + 1) * (D // 2)],
                        transpose=True,
                    )
                xT_tiles[s % n_slab] = xt

            def phaseA_start(s):
                s = s % n_slab
                tlr_tiles[s] = lrpsum.tile([R, T_SLAB], F32, tag="tlr", name="tlr")
                t_sb = tlrpool.tile([R + 1, T_SLAB], BF16, tag="tlr_sb")
                nc.vector.memset(t_sb[R:R + 1, :], 1.0)
                tlr_sb_tiles[s] = t_sb

            def phaseA_st1(s, oc):
                s = s % n_slab
                xT = xT_tiles[s]
                tlr = tlr_tiles[s]
                for kk in range(4):
                    ki = 4 * oc + kk
                    nc.tensor.matmul(
                        tlr[:], v_sb[:, ki, :], xT[:, ki, :],
                        start=(ki == 0), stop=(ki == NK - 1),
                        skip_group_check=True,
                    )

            def phaseA_finish(s):
                s = s % n_slab
                nc.vector.tensor_copy(tlr_sb_tiles[s][:R, :], tlr_tiles[s][:])

            def phaseB(s, o_sbs):
                """block-diag + low-rank(+bias) for slab s, writing o_sbs."""
                s = s % n_slab
                xT = xT_tiles[s]
                tlr_sb = tlr_sb_tiles[s]
                for oc in range(n_oc):
                    yield oc
                    for tcI in range(n_tc):
                        acc = psum.tile([128, OC], F32, tag="acc")
                        for kk in range(4):
                            ki = 4 * oc + kk
                            nc.tensor.matmul(
                                acc[:, (kk // 2) * 256:(kk // 2) * 256 + 256],
                                xT[:, ki, tcI * 128:(tcI + 1) * 128],
                                bt_sb[:, ki, :],
                                start=(kk == 0), stop=False,
                                skip_group_check=True,
                            )
                        nc.tensor.matmul(
                            acc[:], tlr_sb[:, tcI * 128:(tcI + 1) * 128],
                            uaug_sb[:, 4 * oc:4 * oc + 4, :],
                            start=False, stop=True, skip_group_check=True,
                        )
                        eng = nc.scalar if (oc + tcI) % 2 else nc.vector
                        if eng is nc.scalar:
                            nc.scalar.copy(
                                o_sbs[tcI][:, oc * OC:(oc + 1) * OC], acc[:])
                        else:
                            nc.vector.tensor_copy(
                                o_sbs[tcI][:, oc * OC:(oc + 1) * OC], acc[:])

            def store_out(s, o_sbs):
                t0 = (s % n_slab) * T_SLAB
                for tcI in range(n_tc):
                    nc.sync.dma_start(
                        out[t0 + tcI * 128:t0 + (tcI + 1) * 128, :],
                        o_sbs[tcI][:],
                    )

            # ---- prologue: params + slab 0 ----
            v_sb = cpool.tile([128, NK, R], BF16)
            nc.sync.dma_start(v_sb[:], v[:])
            load_xT(0)
            bt_sb = cpool.tile([128, NK, 256], BF16)
            nc.sync.dma_start(bt_sb[:], bt[:])
            uaug_sb = cpool.tile([R + 1, NK, 128], BF16)
            nc.sync.dma_start(uaug_sb[:], uaug[:])

            phaseA_start(0)
            for oc in range(n_oc):
                phaseA_st1(0, oc)
            phaseA_finish(0)

            # ---- software-pipelined steady loop ----
            total = repeats * n_slab
            for it in range(total):
                s = it % n_slab
                nxt = it + 1
                if nxt < total:
                    load_xT(nxt)
                    phaseA_start(nxt)
                o_sbs = [opool.tile([128, D], BF16, tag="osb", name="osb")
                         for _ in range(n_tc)]
                for oc in phaseB(s, o_sbs):
                    if nxt < total:
                        phaseA_st1(nxt, oc)
                store_out(s, o_sbs)
                if nxt < total:
                    phaseA_finish(nxt)
    nc.compile()
    return nc


def check_waits(nc, verbose=True):
    bad = 0
    for fn in nc.m.functions:
        for bb in fn.blocks:
            for ins in bb.instructions:
                tname = type(ins).__name__
                if tname == "InstDrain":
                    continue
                nw = len(ins.sync_info.on_wait) if ins.sync_info else 0
                if tname == "InstEventSemaphore" and nw <= 2:
                    continue
                if nw > 1:
                    bad += 1
                    if verbose:
                        print("MULTI-WAIT", tname, ins.name,
                              [(w.ant_name, w.wait_value) for w in ins.sync_info.on_wait])
    return bad


_NC_CACHE = {}


def _get_nc(t_core, repeats=1):
    key = (t_core, repeats)
    if key not in _NC_CACHE:
        _NC_CACHE[key] = build(t_core, repeats)
    return _NC_CACHE[key]


def make_in_maps(inputs):
    x = np.asarray(inputs["x"], dtype=np.float32).reshape(-1, D)
    blocks = np.asarray(inputs["blocks"], dtype=np.float32)
    U = np.asarray(inputs["U"], dtype=np.float32)
    V = np.asarray(inputs["V"], dtype=np.float32)
    bias = np.asarray(inputs["bias"], dtype=np.float32)
    alpha = float(np.asarray(inputs["alpha"]).reshape(-1)[0])
    t_core = x.shape[0] // N_CORES

    xh = np.ascontiguousarray(x).astype(NPBF)
    # bt[p, 2b+h, o] = alpha * blocks[b, o, h*128+p]
    btn = (alpha * blocks).transpose(2, 0, 1)          # [i, b, o]
    btn = btn.reshape(2, 128, NB, 256).transpose(1, 2, 0, 3)
    btn = np.ascontiguousarray(btn.reshape(128, NK, 256)).astype(NPBF)
    # uaug[r, a, o'] = alpha*U[a*128+o', r] for r<64; bias[a*128+o'] for r=64
    ua = np.concatenate([(alpha * U).T, bias[None, :]], axis=0)
    ua = np.ascontiguousarray(ua.reshape(R + 1, NK, 128)).astype(NPBF)
    # v[p, a, r] = V[a*128+p, r]
    vn = V.reshape(NK, 128, R).transpose(1, 0, 2)
    vn = np.ascontiguousarray(vn).astype(NPBF)

    return [
        {"x": xh[c * t_core:(c + 1) * t_core], "bt": btn, "uaug": ua, "v": vn}
        for c in range(N_CORES)
    ]


def kernel(x, blocks, U, V, bias, alpha):
    batch_dims = x.shape[:-1]
    n_tok = int(np.prod(batch_dims))
    t_core = n_tok // N_CORES
    nc = _get_nc(t_core)
    in_maps = make_in_maps(
        {"x": x, "blocks": blocks, "U": U, "V": V, "bias": bias, "alpha": alpha})
    res = run_bass_kernel_spmd(nc, in_maps, list(range(N_CORES)))
    out = np.concatenate([res.results[c]["out"] for c in range(N_CORES)], axis=0)
    return out.astype(np.float32).reshape(*batch_dims, D)


# revision 14
# speedup vs baseline: 1.1458x; 1.1458x over previous
"""BlockDiagonalLowRankLinear Trainium2 kernel.

y = BlockDiag(blocks) @ x + U @ (V.T @ x), scaled by alpha, plus bias.

Shapes (full problem):
  x      [4, 2048, 4096] f32   -> flattened to [8192, 4096]
  blocks [16, 256, 256]  f32   (per-block [out, in])
  U      [4096, 64] f32, V [4096, 64] f32, bias [4096] f32, alpha [1] f32
  out    [4, 2048, 4096] f32

Sharding: data-parallel over tokens. Each of the 8 cores gets 1024 tokens
and the full (replicated) parameters; outputs are concatenated. No
collectives needed.

The baseline f32 version was DMA-bound (~40MB/core ~= 120us at ~330GB/s).
This version moves all device I/O to bf16 (host converts; rel err ~5e-3
vs the 2e-2 gate) and restructures host-side so the device does nothing
but matmuls:
  - alpha is folded into blocks and U on the host.
  - blocks^T and U^T are pre-transposed on the host into the PE chunk
    layout; bias is appended as a 65th row of U^T so the bias add happens
    inside the low-rank matmul (K=65) for free.
  - x^T is produced by the DMA XBAR transpose (16x128 tiles, bf16) during
    the load - no PE/DVE transposes at all.

Per-core per-slab (256 tokens) steady state:
  DMA: x^T slab load (2 transpose-DMAs), 2 out row-block writes.
  PE : 32 V^T x matmuls (N=256) for the NEXT slab interleaved with
       8x2 accumulation groups of 4 block-diag matmuls (N=256) + 1
       low-rank+bias matmul (K=65, N=512) for THIS slab.
  ACT/DVE: PSUM->SBUF bf16 copies of outputs; DVE copies t_lr to SBUF.
"""

import numpy as np
import ml_dtypes

import concourse.bacc as bacc
import concourse.bass as bass
import concourse.mybir as mybir
import concourse.tile as tile
from concourse.bass_utils import run_bass_kernel_spmd

F32 = mybir.dt.float32
BF16 = mybir.dt.bfloat16
NPBF = ml_dtypes.bfloat16

N_CORES = 8
D = 4096          # in = out features
R = 64            # low rank
NB = 16           # diagonal blocks
NK = D // 128     # 32 i-chunks of 128
T_CORE = 1024     # tokens per core
T_SLAB = 256      # tokens per slab
OC = 512          # output column chunk (one PSUM bank)

# tuning knobs (model-swept)
PSUM_BUFS = 5
XT_BUFS = 2
COPY_ENGINE = "alt"   # alt | act | dve
XT_DMA_SPLIT = 2      # DmaTranspose instructions per slab
ST1_LAG = 2           # bd groups emitted before first st1 group of next slab


def build(t_core: int = T_CORE, repeats: int = 1):
    nc = bacc.Bacc("TRN2", target_bir_lowering=False, debug=False)
    x = nc.declare_dram_parameter("x", [t_core, D], BF16, isOutput=False)
    bt = nc.declare_dram_parameter("bt", [128, NK, 256], BF16, isOutput=False)
    uaug = nc.declare_dram_parameter("uaug", [R + 1, NK, 128], BF16, isOutput=False)
    v = nc.declare_dram_parameter("v", [128, NK, R], BF16, isOutput=False)
    out = nc.declare_dram_parameter("out", [t_core, D], BF16, isOutput=True)

    n_slab = t_core // T_SLAB
    n_tc = T_SLAB // 128          # token chunks per slab
    n_oc = D // OC                # 8 output chunks

    with tile.TileContext(nc) as tc:
        with (
            tc.tile_pool(name="const", bufs=1) as cpool,
            tc.tile_pool(name="psum", bufs=PSUM_BUFS, space="PSUM") as psum,
            tc.tile_pool(name="lrpsum", bufs=2, space="PSUM") as lrpsum,
            tc.tile_pool(name="xT", bufs=XT_BUFS) as xTpool,
            tc.tile_pool(name="tlr", bufs=2) as tlrpool,
            tc.tile_pool(name="opool", bufs=3) as opool,
        ):
            xT_tiles = [None] * n_slab
            tlr_tiles = [None] * n_slab
            tlr_sb_tiles = [None] * n_slab

            def load_xT(s):
                """Transpose-load slab s of x into a fresh xT tile."""
                t0 = (s % n_slab) * T_SLAB
                xt = xTpool.tile([128, NK, T_SLAB], BF16, tag="xT")
                ns = XT_DMA_SPLIT
                for h in range(ns):
                    nc.sync.dma_start(
                        xt[:, h * (NK // ns):(h + 1) * (NK // ns), :],
                        x[t0:t0 + T_SLAB, h * (D // ns):(h + 1) * (D // ns)],
                        transpose=True,
                    )
                xT_tiles[s % n_slab] = xt

            def phaseA_start(s):
                s = s % n_slab
                tlr_tiles[s] = lrpsum.tile([R, T_SLAB], F32, tag="tlr", name="tlr")
                t_sb = tlrpool.tile([R + 1, T_SLAB], BF16, tag="tlr_sb")
                nc.vector.memset(t_sb[R:R + 1, :], 1.0)
                tlr_sb_tiles[s] = t_sb

            def phaseA_st1(s, oc):
                s = s % n_slab
                xT = xT_tiles[s]
                tlr = tlr_tiles[s]
                for kk in range(4):
                    ki = 4 * oc + kk
                    nc.tensor.matmul(
                        tlr[:], v_sb[:, ki, :], xT[:, ki, :],
                        start=(ki == 0), stop=(ki == NK - 1),
                        skip_group_check=True,
                    )

            def phaseA_finish(s):
                s = s % n_slab
                nc.vector.tensor_copy(tlr_sb_tiles[s][:R, :], tlr_tiles[s][:])

            def phaseB_oc(s, oc, o_sbs):
                """block-diag + low-rank(+bias) for o-chunk oc of slab s."""
                s = s % n_slab
                xT = xT_tiles[s]
                tlr_sb = tlr_sb_tiles[s]
                if True:
                    for tcI in range(n_tc):
                        acc = psum.tile([128, OC], F32, tag="acc")
                        for kk in range(4):
                            ki = 4 * oc + kk
                            nc.tensor.matmul(
                                acc[:, (kk // 2) * 256:(kk // 2) * 256 + 256],
                                xT[:, ki, tcI * 128:(tcI + 1) * 128],
                                bt_sb[:, ki, :],
                                start=(kk == 0), stop=False,
                                skip_group_check=True,
                            )
                        nc.tensor.matmul(
                            acc[:], tlr_sb[:, tcI * 128:(tcI + 1) * 128],
                            uaug_sb[:, 4 * oc:4 * oc + 4, :],
                            start=False, stop=True, skip_group_check=True,
                        )
                        use_act = {"alt": (oc + tcI) % 2 == 1,
                                   "act": True, "dve": False}[COPY_ENGINE]
                        if use_act:
                            nc.scalar.copy(
                                o_sbs[tcI][:, oc * OC:(oc + 1) * OC], acc[:])
                        else:
                            nc.vector.tensor_copy(
                                o_sbs[tcI][:, oc * OC:(oc + 1) * OC], acc[:])

            def store_out(s, o_sbs):
                t0 = (s % n_slab) * T_SLAB
                for tcI in range(n_tc):
                    nc.sync.dma_start(
                        out[t0 + tcI * 128:t0 + (tcI + 1) * 128, :],
                        o_sbs[tcI][:],
                    )

            # ---- prologue: params + slab 0 ----
            v_sb = cpool.tile([128, NK, R], BF16)
            nc.sync.dma_start(v_sb[:], v[:])
            load_xT(0)
            bt_sb = cpool.tile([128, NK, 256], BF16)
            nc.sync.dma_start(bt_sb[:], bt[:])
            uaug_sb = cpool.tile([R + 1, NK, 128], BF16)
            nc.sync.dma_start(uaug_sb[:], uaug[:])

            phaseA_start(0)
            for oc in range(n_oc):
                phaseA_st1(0, oc)
            phaseA_finish(0)

            # ---- software-pipelined steady loop ----
            total = repeats * n_slab
            for it in range(total):
                s = it % n_slab
                nxt = it + 1
                if nxt < total:
                    load_xT(nxt)
                    phaseA_start(nxt)
                o_sbs = [opool.tile([128, D], BF16, tag="osb", name="osb")
                         for _ in range(n_tc)]
                # bd groups lead by ST1_LAG so the next slab's transpose-DMA
                # has landed before the first st1 matmul needs it
                for oc in range(n_oc):
                    if nxt < total and oc >= ST1_LAG:
                        phaseA_st1(nxt, oc - ST1_LAG)
                    phaseB_oc(s, oc, o_sbs)
                if nxt < total:
                    for g in range(n_oc - ST1_LAG, n_oc):
                        phaseA_st1(nxt, g)
                store_out(s, o_sbs)
                if nxt < total:
                    phaseA_finish(nxt)
    nc.compile()
    return nc


def check_waits(nc, verbose=True):
    bad = 0
    for fn in nc.m.functions:
        for bb in fn.blocks:
            for ins in bb.instructions:
                tname = type(ins).__name__
                if tname == "InstDrain":
                    continue
                nw = len(ins.sync_info.on_wait) if ins.sync_info else 0
                if tname == "InstEventSemaphore" and nw <= 2:
                    continue
                if nw > 1:
                    bad += 1
                    if verbose:
                        print("MULTI-WAIT", tname, ins.name,
                              [(w.ant_name, w.wait_value) for w in ins.sync_info.on_wait])
    return bad


_NC_CACHE = {}


def _get_nc(t_core, repeats=1):
    key = (t_core, repeats)
    if key not in _NC_CACHE:
        _NC_CACHE[key] = build(t_core, repeats)
    return _NC_CACHE[key]


def make_in_maps(inputs):
    x = np.asarray(inputs["x"], dtype=np.float32).reshape(-1, D)
    blocks = np.asarray(inputs["blocks"], dtype=np.float32)
    U = np.asarray(inputs["U"], dtype=np.float32)
    V = np.asarray(inputs["V"], dtype=np.float32)
    bias = np.asarray(inputs["bias"], dtype=np.float32)
    alpha = float(np.asarray(inputs["alpha"]).reshape(-1)[0])
    t_core = x.shape[0] // N_CORES

    xh = np.ascontiguousarray(x).astype(NPBF)
    # bt[p, 2b+h, o] = alpha * blocks[b, o, h*128+p]
    btn = (alpha * blocks).transpose(2, 0, 1)          # [i, b, o]
    btn = btn.reshape(2, 128, NB, 256).transpose(1, 2, 0, 3)
    btn = np.ascontiguousarray(btn.reshape(128, NK, 256)).astype(NPBF)
    # uaug[r, a, o'] = alpha*U[a*128+o', r] for r<64; bias[a*128+o'] for r=64
    ua = np.concatenate([(alpha * U).T, bias[None, :]], axis=0)
    ua = np.ascontiguousarray(ua.reshape(R + 1, NK, 128)).astype(NPBF)
    # v[p, a, r] = V[a*128+p, r]
    vn = V.reshape(NK, 128, R).transpose(1, 0, 2)
    vn = np.ascontiguousarray(vn).astype(NPBF)

    return [
        {"x": xh[c * t_core:(c + 1) * t_core], "bt": btn, "uaug": ua, "v": vn}
        for c in range(N_CORES)
    ]


def kernel(x, blocks, U, V, bias, alpha):
    batch_dims = x.shape[:-1]
    n_tok = int(np.prod(batch_dims))
    t_core = n_tok // N_CORES
    nc = _get_nc(t_core)
    in_maps = make_in_maps(
        {"x": x, "blocks": blocks, "U": U, "V": V, "bias": bias, "alpha": alpha})
    res = run_bass_kernel_spmd(nc, in_maps, list(range(N_CORES)))
    out = np.concatenate([res.results[c]["out"] for c in range(N_CORES)], axis=0)
    return out.astype(np.float32).reshape(*batch_dims, D)
